# revision 14
# baseline (speedup 1.0000x reference)
"""ConvShapeletNet Trainium2 kernel — fp16 fold-order staging, grouped tiles.

Math (per batch row b, channel c):
  xb = x.reshape(B, C, L)                    # pure view: row r=(b,c) is 8192 contiguous floats
  win[o]  = sum(xb[r, o*286 : o*286+1146])   # o in [0, 24): only 24 of 25 conv outputs
                                             # survive MaxPool1d(3) (floor(25/3)*3 = 24)
  y       = (win + conv_bias[c])^2
  pooled  = max(-y over window 3) = -(min y over window 3)    -> (B, 10, 8)
  out     = pooled.reshape(B, 80) @ fc_w.T + fc_b

Only x[:, 0:7724] is ever read (23*286 + 1146 = 7724), and the 2e-2 rel-err
budget allows fp16 input staging — halving HBM traffic, the sole bottleneck.

Block sums: win[o] = L2[o]+L2[o+1]+L2[o+2]+L2[o+3] + x[286(o+4)] + x[286(o+4)+1]
with L2[k] = sum(x[286k : 286k+286]), k in [0, 27).

DVE tensor_reduce is capped at 1x mode (1 elem/cycle/partition), but
tensor_tensor add runs 2x_1p on dense 16-bit data. So the host pre-permutes
each zero-padded 288-element block into fold order — element j = a0*144 +
a1*72 + a2*36 + a3*18 + r of block k goes to column
((((a0*2+a1)*2+a2)*2+a3)*27 + k)*18 + r — making every fold level a single
CONTIGUOUS halves-add. Four 2x folds (7776 -> 486) + an 18-wide 1x reduce
give L2 in f32.

Each window's two extras and its channel's conv bias are staged as a
[24, 3] appendix; one [*, 24, 3] reduce yields ex[o] = x0 + x1 + bias, so
y = Square(win) needs no per-tile bias operand. That removes the per-tile
ACT bias table and lets DVE ops batch over TILE GROUPS of [3, 2] row-tiles
(fixed ~0.4us/op DVE overhead was ~30% of runtime at 1 tile/op). Fold
buffers use bufs=1: consecutive groups' WAR hazards are same-engine
(DVE-serial) so no double buffering is needed.

Sharding: pure data parallel, batch 512 -> 64 per core across 8 cores.
"""

import numpy as np
from contextlib import ExitStack

import concourse.bass as bass
import concourse.tile as tile
from concourse import bacc, masks, mybir
from concourse.bass_utils import run_bass_kernel_spmd

F32 = mybir.dt.float32
F16 = mybir.dt.float16

N_CORES = 8
B_FULL = 512
B_SH = B_FULL // N_CORES     # 64 batches per core
C = 10                       # variates / conv groups
L = 8192
ROWS = B_SH * C              # 640 rows per core
STRIDE = 286
NB = 27                      # 286-blocks summed (win[o] needs blocks o..o+3, o<24)
NR = 18                      # residual reduce width after 4 folds
MAIN_W = 16 * NB * NR        # 7776 fold-ordered columns
EX_OFF = MAIN_W
N_EX = 24                    # extras triples (x0, x1, bias) per window
XT_W = MAIN_W + 3 * N_EX     # 7848 staged columns per row
L_OUT = 24                   # windows that survive pooling
L_P = 8
POOLK = 3
N_CLASSES = 10
TILE_P = 128
N_TILES = ROWS // TILE_P     # 5
GROUPS = (2, 2, 1)           # row-tiles per DVE op batch
GMAX = max(GROUPS)


def build_nc(reps=1, x_bufs=4, strided_out=False):
    """Build the per-core program. reps>1 unrolls the whole computation
    multiple times inside one NEFF (identical result; used for timing)."""
    nc = bacc.Bacc("TRN2", target_bir_lowering=False, debug=False,
                   num_devices=N_CORES)

    x = nc.dram_tensor("x", [ROWS, XT_W], F16, kind="ExternalInput")
    fc_w = nc.dram_tensor("fc_w", [N_CLASSES, C * L_P], F32, kind="ExternalInput")
    fc_b = nc.dram_tensor("fc_b", [N_CLASSES], F32, kind="ExternalInput")
    # out is stored transposed (n, b): the final DMA is then 10 contiguous
    # 256B lines instead of 640 strided 4B segments (descriptor-bound on HW).
    # The host transposes during unshard.
    out_shape = [B_SH, N_CLASSES] if strided_out else [N_CLASSES, B_SH]
    out = nc.dram_tensor("out", out_shape, F32, kind="ExternalOutput")

    with tile.TileContext(nc) as tc, ExitStack() as ctx:
        const = ctx.enter_context(tc.tile_pool(name="const", bufs=1))
        xpool = ctx.enter_context(tc.tile_pool(name="x", bufs=x_bufs))
        fbpool = ctx.enter_context(tc.tile_pool(name="fb", bufs=1))
        work = ctx.enter_context(tc.tile_pool(name="work", bufs=2))
        pooledp = ctx.enter_context(tc.tile_pool(name="pooledp", bufs=2))
        tpsum = ctx.enter_context(tc.tile_pool(name="tpsum", bufs=3, space="PSUM"))
        opsum = ctx.enter_context(tc.tile_pool(name="opsum", bufs=2, space="PSUM"))
        mtp = ctx.enter_context(tc.tile_pool(name="mtp", bufs=2))

        # ---- constants (once) ----
        ident = const.tile([TILE_P, TILE_P], F32)
        masks.make_identity(nc, ident[:])

        # w8[k, n*10+c] = fc_w[n, c*8+k], built without any gather DMA:
        # load fc_w contiguously, then one tiny PE transpose per channel.
        fw = const.tile([N_CLASSES, C * L_P], F32)
        nc.sync.dma_start(out=fw[:], in_=fc_w.ap())
        w8 = const.tile([L_P, N_CLASSES * C], F32)
        w8v3 = w8[:].rearrange("k (n c) -> k n c", c=C)

        fcb = const.tile([N_CLASSES, 1], F32)
        nc.sync.dma_start(out=fcb[:], in_=fc_b.ap().unsqueeze(1))

        wps = ctx.enter_context(tc.tile_pool(name="wps", bufs=1, space="PSUM"))
        for c in range(C):
            wt = wps.tile([L_P, N_CLASSES], F32, tag="wt")
            nc.tensor.transpose(wt[:], fw[:, c * L_P:(c + 1) * L_P],
                                ident[0:N_CLASSES, 0:N_CLASSES])
            nc.scalar.copy(w8v3[:, :, c], wt[:])

        xap = x.ap()
        for _ in range(reps):
            # transposed pooled accumulator: mt[k, r] = pooled[r, k]
            mt = mtp.tile([L_P, ROWS], F32, tag="mt")
            t0 = 0
            for gi, G in enumerate(GROUPS):
                # one [128, G, 7848] load: row-tiles t0..t0+G share partitions.
                # Alternate the two HWDGE rings (SP / ACT) so per-DMA setup
                # on one ring hides behind the other's transfer.
                xt = xpool.tile([TILE_P, GMAX * XT_W], F16, tag="xt")
                xg = xt[:].rearrange("p (g w) -> p g w", w=XT_W)[:, 0:G, :]
                dma_eng = nc.sync if gi % 2 == 0 else nc.scalar
                dma_eng.dma_start(
                    out=xg,
                    in_=xap[t0 * TILE_P:(t0 + G) * TILE_P, :].rearrange(
                        "(g p) w -> p g w", p=TILE_P))

                # fp16 fold tree on DVE, every level one contiguous-per-group
                # halves-add in 2x_1p: 7776 -> 3888 -> 1944 -> 972 -> 486.
                # The extras reduce is issued right after fold1 so the xt
                # buffer (read only by these two) frees as early as possible.
                fb1 = fbpool.tile([TILE_P, GMAX * 3888], F16, tag="fb1")
                f1 = fb1[:].rearrange("p (g w) -> p g w", w=3888)[:, 0:G, :]
                nc.vector.tensor_add(f1, xg[:, :, 0:3888], xg[:, :, 3888:7776])

                # appendix: ex[o] = x[286(o+4)] + x[286(o+4)+1] + conv_bias[c]
                ex = work.tile([TILE_P, GMAX * N_EX], F32, tag="ex")
                exg = ex[:].rearrange("p (g o) -> p g o", o=N_EX)[:, 0:G, :]
                nc.vector.reduce_sum(
                    exg, xg[:, :, EX_OFF:XT_W].rearrange(
                        "p g (o j) -> p g o j", j=3),
                    axis=mybir.AxisListType.X)

                fb2 = fbpool.tile([TILE_P, GMAX * 1944], F16, tag="fb2")
                f2 = fb2[:].rearrange("p (g w) -> p g w", w=1944)[:, 0:G, :]
                nc.vector.tensor_add(f2, f1[:, :, 0:1944], f1[:, :, 1944:3888])
                fb3 = fbpool.tile([TILE_P, GMAX * 972], F16, tag="fb3")
                f3 = fb3[:].rearrange("p (g w) -> p g w", w=972)[:, 0:G, :]
                nc.vector.tensor_add(f3, f2[:, :, 0:972], f2[:, :, 972:1944])
                fb4 = fbpool.tile([TILE_P, GMAX * 486], F16, tag="fb4")
                f4 = fb4[:].rearrange("p (g w) -> p g w", w=486)[:, 0:G, :]
                nc.vector.tensor_add(f4, f3[:, :, 0:486], f3[:, :, 486:972])
                l2 = work.tile([TILE_P, GMAX * NB], F32, tag="l2")
                l2g = l2[:].rearrange("p (g k) -> p g k", k=NB)[:, 0:G, :]
                nc.vector.reduce_sum(
                    l2g, f4.rearrange("p g (k j) -> p g k j", j=NR),
                    axis=mybir.AxisListType.X)

                # win[o] = l2[o]+l2[o+1]+l2[o+2]+l2[o+3] + ex[o] on idle GPSIMD
                t1 = work.tile([TILE_P, GMAX * L_OUT], F32, tag="t1")
                t1g = t1[:].rearrange("p (g o) -> p g o", o=L_OUT)[:, 0:G, :]
                nc.gpsimd.tensor_add(t1g, l2g[:, :, 0:24], l2g[:, :, 1:25])
                t2 = work.tile([TILE_P, GMAX * L_OUT], F32, tag="t2")
                t2g = t2[:].rearrange("p (g o) -> p g o", o=L_OUT)[:, 0:G, :]
                nc.gpsimd.tensor_add(t2g, l2g[:, :, 2:26], l2g[:, :, 3:27])
                t3 = work.tile([TILE_P, GMAX * L_OUT], F32, tag="t3")
                t3g = t3[:].rearrange("p (g o) -> p g o", o=L_OUT)[:, 0:G, :]
                nc.gpsimd.tensor_add(t3g, t1g, t2g)
                win = work.tile([TILE_P, GMAX * L_OUT], F32, tag="win")
                wing = win[:].rearrange("p (g o) -> p g o", o=L_OUT)[:, 0:G, :]
                nc.gpsimd.tensor_add(wing, t3g, exg)

                # y = win^2 on ScalarE (bias already in win via the appendix);
                # pooled = -(min_3 y) = max_3(-y) on DVE
                y = work.tile([TILE_P, GMAX * L_OUT], F32, tag="y")
                yg = y[:].rearrange("p (g o) -> p g o", o=L_OUT)[:, 0:G, :]
                nc.scalar.activation(yg, wing,
                                     mybir.ActivationFunctionType.Square,
                                     bias=0.0, scale=1.0)
                pooled = pooledp.tile([TILE_P, GMAX * L_P], F32, tag="pooled")
                pg = pooled[:].rearrange("p (g k) -> p g k", k=L_P)[:, 0:G, :]
                nc.vector.tensor_reduce(
                    pg, yg.rearrange("p g (k j) -> p g k j", j=POOLK),
                    axis=mybir.AxisListType.X, op=mybir.AluOpType.min, negate=True)

                # PE transposes (128, 8) -> (8, 128) per tile (PSUM reads
                # can't start at a nonzero partition), stash into mt
                for g in range(G):
                    pt = tpsum.tile([L_P, TILE_P], F32, tag="pt")
                    nc.tensor.transpose(
                        pt[:], pooled[:, g * L_P:(g + 1) * L_P], ident[:])
                    nc.scalar.copy(
                        mt[:, (t0 + g) * TILE_P:(t0 + g + 1) * TILE_P], pt[:])
                t0 += G

            # FC: out[n, b] = sum_c sum_k w8[k, n*10+c] * mt[k, b*10+c]
            ops = opsum.tile([N_CLASSES, B_SH], F32, tag="ops")
            mtv = mt[:].rearrange("k (b c) -> k b c", c=C)
            w8v = w8[:].rearrange("k (n c) -> k n c", c=C)
            for c in range(C):
                nc.tensor.matmul(ops[:], w8v[:, :, c],
                                 mtv[:, :, c], start=(c == 0), stop=(c == C - 1))
            outsb = mtp.tile([N_CLASSES, B_SH], F32, tag="outsb")
            nc.scalar.add(outsb[:], ops[:], fcb[:, 0:1])
            # out DMA on the ACT HWDGE ring: keeps the sync ring streaming x
            nc.scalar.dma_start(
                out=out.ap().transpose([1, 0]) if strided_out else out.ap(),
                in_=outsb[:])

    nc.compile()
    return nc


_NC_CACHE = None


def _get_nc():
    global _NC_CACHE
    if _NC_CACHE is None:
        _NC_CACHE = build_nc()
    return _NC_CACHE


# extras column indices in the original row: x[286(o+4)], x[286(o+4)+1]
_EX_IDX = np.add.outer(STRIDE * (np.arange(N_EX) + 4), np.arange(2))


def _stage_x(shard, conv_bias):
    """(B_SH, 8192, 10) f32 -> (640, 7848) fp16: fold-ordered main block
    plus a [24, 3] appendix of (x0, x1, conv_bias[c]) per window."""
    xr = np.ascontiguousarray(shard).reshape(ROWS, L)
    blocks = np.zeros((ROWS, NB, 16 * NR), dtype=np.float16)
    blocks[:, :, 0:STRIDE] = xr[:, 0:NB * STRIDE].reshape(ROWS, NB, STRIDE)
    # (ROWS, k, a0, a1, a2, a3, r) -> (ROWS, a0, a1, a2, a3, k, r)
    perm = blocks.reshape(ROWS, NB, 2, 2, 2, 2, NR).transpose(0, 2, 3, 4, 5, 1, 6)
    out = np.empty((ROWS, XT_W), dtype=np.float16)
    out[:, 0:MAIN_W] = perm.reshape(ROWS, MAIN_W)
    app = out[:, EX_OFF:XT_W].reshape(ROWS, N_EX, 3)
    app[:, :, 0:2] = xr[:, _EX_IDX]
    app[:, :, 2] = np.tile(conv_bias.astype(np.float16), B_SH)[:, None]
    return out


def make_in_maps(x, conv_bias, fc_w, fc_b):
    x = np.asarray(x, dtype=np.float32)
    conv_bias = np.asarray(conv_bias, dtype=np.float32)
    fc_w = np.asarray(fc_w, dtype=np.float32)
    fc_b = np.asarray(fc_b, dtype=np.float32)
    in_maps = []
    for i in range(N_CORES):
        in_maps.append({
            "x": _stage_x(x[i * B_SH:(i + 1) * B_SH], conv_bias),
            "fc_w": fc_w,
            "fc_b": fc_b,
        })
    return in_maps


def kernel(x, conv_bias, fc_w, fc_b, trace=False):
    nc = _get_nc()
    in_maps = make_in_maps(x, conv_bias, fc_w, fc_b)
    res = run_bass_kernel_spmd(nc, in_maps, list(range(N_CORES)), trace=trace)
    kernel.last_result = res
    # per-core output is (n_classes, batch_shard): transpose while unsharding
    out = np.concatenate([res.results[i]["out"].T for i in range(N_CORES)], axis=0)
    return np.ascontiguousarray(out, dtype=np.float32)


# revision 15
# speedup vs baseline: 1.2236x; 1.2236x over previous
"""ConvShapeletNet Trainium2 kernel — fp16 fold-order staging, grouped tiles.

Math (per batch row b, channel c):
  xb = x.reshape(B, C, L)                    # pure view: row r=(b,c) is 8192 contiguous floats
  win[o]  = sum(xb[r, o*286 : o*286+1146])   # o in [0, 24): only 24 of 25 conv outputs
                                             # survive MaxPool1d(3) (floor(25/3)*3 = 24)
  y       = (win + conv_bias[c])^2
  pooled  = max(-y over window 3) = -(min y over window 3)    -> (B, 10, 8)
  out     = pooled.reshape(B, 80) @ fc_w.T + fc_b

Only x[:, 0:7724] is ever read (23*286 + 1146 = 7724), and the 2e-2 rel-err
budget allows fp16 input staging — halving HBM traffic, the sole bottleneck.

Block sums: win[o] = L2[o]+L2[o+1]+L2[o+2]+L2[o+3] + x[286(o+4)] + x[286(o+4)+1]
with L2[k] = sum(x[286k : 286k+286]), k in [0, 27).

DVE tensor_reduce is capped at 1x mode (1 elem/cycle/partition), but
tensor_tensor add runs 2x_1p on dense 16-bit data. So the host pre-permutes
each zero-padded 288-element block into fold order — element j = a0*144 +
a1*72 + a2*36 + a3*18 + r of block k goes to column
((((a0*2+a1)*2+a2)*2+a3)*27 + k)*18 + r — making every fold level a single
CONTIGUOUS halves-add. Four 2x folds (7776 -> 486) + an 18-wide 1x reduce
give L2 in f32.

Each window's two extras and its channel's conv bias are staged as a
[24, 3] appendix; one [*, 24, 3] reduce yields ex[o] = x0 + x1 + bias, so
y = Square(win) needs no per-tile bias operand. That removes the per-tile
ACT bias table and lets DVE ops batch over TILE GROUPS of [3, 2] row-tiles
(fixed ~0.4-0.7us/op DVE overhead was ~30% of runtime at 1 tile/op). Fold
buffers use bufs=1: consecutive groups' WAR hazards are same-engine
(DVE-serial) so no double buffering is needed.

Sharding: pure data parallel, batch 512 -> 64 per core across 8 cores.
"""

import numpy as np
from contextlib import ExitStack

import concourse.bass as bass
import concourse.tile as tile
from concourse import bacc, masks, mybir
from concourse.bass_utils import run_bass_kernel_spmd

F32 = mybir.dt.float32
F16 = mybir.dt.float16

N_CORES = 8
B_FULL = 512
B_SH = B_FULL // N_CORES     # 64 batches per core
C = 10                       # variates / conv groups
L = 8192
ROWS = B_SH * C              # 640 rows per core
STRIDE = 286
NB = 27                      # 286-blocks summed (win[o] needs blocks o..o+3, o<24)
NR = 18                      # residual reduce width after 4 folds
MAIN_W = 16 * NB * NR        # 7776 fold-ordered columns
EX_OFF = MAIN_W
N_EX = 24                    # extras triples (x0, x1, bias) per window
XT_W = MAIN_W + 3 * N_EX     # 7848 staged columns per row
L_OUT = 24                   # windows that survive pooling
L_P = 8
POOLK = 3
N_CLASSES = 10
TILE_P = 128
N_TILES = ROWS // TILE_P     # 5
GROUPS = (3, 2)              # row-tiles per DVE op batch
GMAX = max(GROUPS)


def build_nc(reps=1, x_bufs=3, strided_out=False):
    """Build the per-core program. reps>1 unrolls the whole computation
    multiple times inside one NEFF (identical result; used for timing)."""
    nc = bacc.Bacc("TRN2", target_bir_lowering=False, debug=False,
                   num_devices=N_CORES)

    x = nc.dram_tensor("x", [ROWS, XT_W], F16, kind="ExternalInput")
    fc_w = nc.dram_tensor("fc_w", [N_CLASSES, C * L_P], F32, kind="ExternalInput")
    fc_b = nc.dram_tensor("fc_b", [N_CLASSES], F32, kind="ExternalInput")
    # out is stored transposed (n, b): the final DMA is then 10 contiguous
    # 256B lines instead of 640 strided 4B segments (descriptor-bound on HW).
    # The host transposes during unshard.
    out_shape = [B_SH, N_CLASSES] if strided_out else [N_CLASSES, B_SH]
    out = nc.dram_tensor("out", out_shape, F32, kind="ExternalOutput")

    with tile.TileContext(nc) as tc, ExitStack() as ctx:
        const = ctx.enter_context(tc.tile_pool(name="const", bufs=1))
        xpool = ctx.enter_context(tc.tile_pool(name="x", bufs=x_bufs))
        fbpool = ctx.enter_context(tc.tile_pool(name="fb", bufs=1))
        work = ctx.enter_context(tc.tile_pool(name="work", bufs=2))
        pooledp = ctx.enter_context(tc.tile_pool(name="pooledp", bufs=2))
        tpsum = ctx.enter_context(tc.tile_pool(name="tpsum", bufs=3, space="PSUM"))
        opsum = ctx.enter_context(tc.tile_pool(name="opsum", bufs=2, space="PSUM"))
        mtp = ctx.enter_context(tc.tile_pool(name="mtp", bufs=2))

        # ---- constants (once) ----
        ident = const.tile([TILE_P, TILE_P], F32)
        masks.make_identity(nc, ident[:])

        # w8[k, n*10+c] = fc_w[n, c*8+k], built without any gather DMA:
        # load fc_w contiguously, then one tiny PE transpose per channel.
        fw = const.tile([N_CLASSES, C * L_P], F32)
        nc.sync.dma_start(out=fw[:], in_=fc_w.ap())
        w8 = const.tile([L_P, N_CLASSES * C], F32)
        w8v3 = w8[:].rearrange("k (n c) -> k n c", c=C)

        fcb = const.tile([N_CLASSES, 1], F32)
        nc.sync.dma_start(out=fcb[:], in_=fc_b.ap().unsqueeze(1))

        wps = ctx.enter_context(tc.tile_pool(name="wps", bufs=1, space="PSUM"))
        for c in range(C):
            wt = wps.tile([L_P, N_CLASSES], F32, tag="wt")
            nc.tensor.transpose(wt[:], fw[:, c * L_P:(c + 1) * L_P],
                                ident[0:N_CLASSES, 0:N_CLASSES])
            nc.scalar.copy(w8v3[:, :, c], wt[:])

        xap = x.ap()
        for _ in range(reps):
            # transposed pooled accumulator: mt[k, r] = pooled[r, k]
            mt = mtp.tile([L_P, ROWS], F32, tag="mt")
            t0 = 0
            for gi, G in enumerate(GROUPS):
                # one [128, G, 7848] load: row-tiles t0..t0+G share partitions.
                # Alternate the two HWDGE rings (SP / ACT) so per-DMA setup
                # on one ring hides behind the other's transfer.
                xt = xpool.tile([TILE_P, GMAX * XT_W], F16, tag="xt")
                xg = xt[:].rearrange("p (g w) -> p g w", w=XT_W)[:, 0:G, :]
                dma_eng = nc.sync if gi % 2 == 0 else nc.scalar
                dma_eng.dma_start(
                    out=xg,
                    in_=xap[t0 * TILE_P:(t0 + G) * TILE_P, :].rearrange(
                        "(g p) w -> p g w", p=TILE_P))

                # fp16 fold tree on DVE, every level one contiguous-per-group
                # halves-add in 2x_1p: 7776 -> 3888 -> 1944 -> 972 -> 486.
                # The extras reduce is issued right after fold1 so the xt
                # buffer (read only by these two) frees as early as possible.
                fb1 = fbpool.tile([TILE_P, GMAX * 3888], F16, tag="fb1")
                f1 = fb1[:].rearrange("p (g w) -> p g w", w=3888)[:, 0:G, :]
                nc.vector.tensor_add(f1, xg[:, :, 0:3888], xg[:, :, 3888:7776])

                # appendix: ex[o] = x[286(o+4)] + x[286(o+4)+1] + conv_bias[c]
                ex = work.tile([TILE_P, GMAX * N_EX], F32, tag="ex")
                exg = ex[:].rearrange("p (g o) -> p g o", o=N_EX)[:, 0:G, :]
                nc.vector.reduce_sum(
                    exg, xg[:, :, EX_OFF:XT_W].rearrange(
                        "p g (o j) -> p g o j", j=3),
                    axis=mybir.AxisListType.X)

                fb2 = fbpool.tile([TILE_P, GMAX * 1944], F16, tag="fb2")
                f2 = fb2[:].rearrange("p (g w) -> p g w", w=1944)[:, 0:G, :]
                nc.vector.tensor_add(f2, f1[:, :, 0:1944], f1[:, :, 1944:3888])
                fb3 = fbpool.tile([TILE_P, GMAX * 972], F16, tag="fb3")
                f3 = fb3[:].rearrange("p (g w) -> p g w", w=972)[:, 0:G, :]
                nc.vector.tensor_add(f3, f2[:, :, 0:972], f2[:, :, 972:1944])
                fb4 = fbpool.tile([TILE_P, GMAX * 486], F16, tag="fb4")
                f4 = fb4[:].rearrange("p (g w) -> p g w", w=486)[:, 0:G, :]
                nc.vector.tensor_add(f4, f3[:, :, 0:486], f3[:, :, 486:972])
                l2 = work.tile([TILE_P, GMAX * NB], F32, tag="l2")
                l2g = l2[:].rearrange("p (g k) -> p g k", k=NB)[:, 0:G, :]
                nc.vector.reduce_sum(
                    l2g, f4.rearrange("p g (k j) -> p g k j", j=NR),
                    axis=mybir.AxisListType.X)

                # win[o] = l2[o]+l2[o+1]+l2[o+2]+l2[o+3] + ex[o] on idle GPSIMD
                t1 = work.tile([TILE_P, GMAX * L_OUT], F32, tag="t1")
                t1g = t1[:].rearrange("p (g o) -> p g o", o=L_OUT)[:, 0:G, :]
                nc.gpsimd.tensor_add(t1g, l2g[:, :, 0:24], l2g[:, :, 1:25])
                t2 = work.tile([TILE_P, GMAX * L_OUT], F32, tag="t2")
                t2g = t2[:].rearrange("p (g o) -> p g o", o=L_OUT)[:, 0:G, :]
                nc.gpsimd.tensor_add(t2g, l2g[:, :, 2:26], l2g[:, :, 3:27])
                t3 = work.tile([TILE_P, GMAX * L_OUT], F32, tag="t3")
                t3g = t3[:].rearrange("p (g o) -> p g o", o=L_OUT)[:, 0:G, :]
                nc.gpsimd.tensor_add(t3g, t1g, t2g)
                win = work.tile([TILE_P, GMAX * L_OUT], F32, tag="win")
                wing = win[:].rearrange("p (g o) -> p g o", o=L_OUT)[:, 0:G, :]
                nc.gpsimd.tensor_add(wing, t3g, exg)

                # y = win^2 on ScalarE (bias already in win via the appendix);
                # pooled = -(min_3 y) = max_3(-y) on DVE
                y = work.tile([TILE_P, GMAX * L_OUT], F32, tag="y")
                yg = y[:].rearrange("p (g o) -> p g o", o=L_OUT)[:, 0:G, :]
                nc.scalar.activation(yg, wing,
                                     mybir.ActivationFunctionType.Square,
                                     bias=0.0, scale=1.0)
                pooled = pooledp.tile([TILE_P, GMAX * L_P], F32, tag="pooled")
                pg = pooled[:].rearrange("p (g k) -> p g k", k=L_P)[:, 0:G, :]
                nc.vector.tensor_reduce(
                    pg, yg.rearrange("p g (k j) -> p g k j", j=POOLK),
                    axis=mybir.AxisListType.X, op=mybir.AluOpType.min, negate=True)

                # PE transposes (128, 8) -> (8, 128) per tile (PSUM reads
                # can't start at a nonzero partition), stash into mt
                for g in range(G):
                    pt = tpsum.tile([L_P, TILE_P], F32, tag="pt")
                    nc.tensor.transpose(
                        pt[:], pooled[:, g * L_P:(g + 1) * L_P], ident[:])
                    nc.scalar.copy(
                        mt[:, (t0 + g) * TILE_P:(t0 + g + 1) * TILE_P], pt[:])
                t0 += G

            # FC: out[n, b] = sum_c sum_k w8[k, n*10+c] * mt[k, b*10+c]
            ops = opsum.tile([N_CLASSES, B_SH], F32, tag="ops")
            mtv = mt[:].rearrange("k (b c) -> k b c", c=C)
            w8v = w8[:].rearrange("k (n c) -> k n c", c=C)
            for c in range(C):
                nc.tensor.matmul(ops[:], w8v[:, :, c],
                                 mtv[:, :, c], start=(c == 0), stop=(c == C - 1))
            outsb = mtp.tile([N_CLASSES, B_SH], F32, tag="outsb")
            nc.scalar.add(outsb[:], ops[:], fcb[:, 0:1])
            # out DMA on the ACT HWDGE ring: keeps the sync ring streaming x
            nc.scalar.dma_start(
                out=out.ap().transpose([1, 0]) if strided_out else out.ap(),
                in_=outsb[:])

    nc.compile()
    return nc


_NC_CACHE = None


def _get_nc():
    global _NC_CACHE
    if _NC_CACHE is None:
        _NC_CACHE = build_nc()
    return _NC_CACHE


# extras column indices in the original row: x[286(o+4)], x[286(o+4)+1]
_EX_IDX = np.add.outer(STRIDE * (np.arange(N_EX) + 4), np.arange(2))


def _stage_x(shard, conv_bias):
    """(B_SH, 8192, 10) f32 -> (640, 7848) fp16: fold-ordered main block
    plus a [24, 3] appendix of (x0, x1, conv_bias[c]) per window."""
    xr = np.ascontiguousarray(shard).reshape(ROWS, L)
    blocks = np.zeros((ROWS, NB, 16 * NR), dtype=np.float16)
    blocks[:, :, 0:STRIDE] = xr[:, 0:NB * STRIDE].reshape(ROWS, NB, STRIDE)
    # (ROWS, k, a0, a1, a2, a3, r) -> (ROWS, a0, a1, a2, a3, k, r)
    perm = blocks.reshape(ROWS, NB, 2, 2, 2, 2, NR).transpose(0, 2, 3, 4, 5, 1, 6)
    out = np.empty((ROWS, XT_W), dtype=np.float16)
    out[:, 0:MAIN_W] = perm.reshape(ROWS, MAIN_W)
    app = out[:, EX_OFF:XT_W].reshape(ROWS, N_EX, 3)
    app[:, :, 0:2] = xr[:, _EX_IDX]
    app[:, :, 2] = np.tile(conv_bias.astype(np.float16), B_SH)[:, None]
    return out


def make_in_maps(x, conv_bias, fc_w, fc_b):
    x = np.asarray(x, dtype=np.float32)
    conv_bias = np.asarray(conv_bias, dtype=np.float32)
    fc_w = np.asarray(fc_w, dtype=np.float32)
    fc_b = np.asarray(fc_b, dtype=np.float32)
    in_maps = []
    for i in range(N_CORES):
        in_maps.append({
            "x": _stage_x(x[i * B_SH:(i + 1) * B_SH], conv_bias),
            "fc_w": fc_w,
            "fc_b": fc_b,
        })
    return in_maps


def kernel(x, conv_bias, fc_w, fc_b, trace=False):
    nc = _get_nc()
    in_maps = make_in_maps(x, conv_bias, fc_w, fc_b)
    res = run_bass_kernel_spmd(nc, in_maps, list(range(N_CORES)), trace=trace)
    kernel.last_result = res
    # per-core output is (n_classes, batch_shard): transpose while unsharding
    out = np.concatenate([res.results[i]["out"].T for i in range(N_CORES)], axis=0)
    return np.ascontiguousarray(out, dtype=np.float32)
